# revision 3
# baseline (speedup 1.0000x reference)
"""LAGCNII full-device kernel for 8 TRN2 NeuronCores (v3).

Node-sharded (12544 padded rows/core, 98 windows of 128). Per layer l
(W'_l = (1-b)I + b*W_l):
  u = h @ (0.9 W'_l)        dense pass, hT kept resident in SBUF
  AllGather(u) -> uag       one collective per layer, rank-major layout
  router: per dst window w the incoming source rows are pre-grouped by
    source HBM-pair (uag rows are pair-contiguous, so int16 dma_gather
    indices address one pair region); gathered tiles reduce into PSUM
    via one-hot matmuls; the c = h0 @ (0.1 W'_l) term accumulates into
    the same PSUM; relu -> h; PE-transpose -> hT (SBUF, single buffer)
Tiles are a uniform (window x pair) grid: exactly 1 gather tile per
(w, p), 4 per window, 392 per layer, so the program is identical across
cores (SPMD) and only the gather indices / one-hot payloads differ.
Gathers are batched with dma_gather per (8-window block, pair):
52 calls/layer instead of 302 indirect DMAs. One-hots stream from DRAM
pre-transposed ([128 slot, tile*128+dst] layout) in 13 big DMAs/layer.
h never round-trips DRAM. Output fp32.
"""
import numpy as np
import ml_dtypes

BF16 = ml_dtypes.bfloat16

N = 100000
E = 300000
PER = 12500
NL = 12544           # 98*128
W = 98
CH = 256
DH = 128
C = 40
L = 8
NC = 8
NP = 4               # HBM pairs
THETA = 0.5

BW = 8                                    # windows per block
BLOCKS = [list(range(b, min(b + BW, W))) for b in range(0, W, BW)]
NB = len(BLOCKS)                          # 13
T_TOTAL = NP * W                          # 392 tiles per layer
# global tile index: blocks outer, then pair, then window-within-block
_TB = []                                  # per block: base tile index
_t = 0
for _b in BLOCKS:
    _TB.append(_t)
    _t += NP * len(_b)


def tile_index(b, p, wi):
    return _TB[b] + p * len(BLOCKS[b]) + wi


_cached = {}


def _pack_windows(deg_pair):
    """Assign PER nodes to W windows of exactly 128 slots, keeping the
    per-(window, pair) in-degree load <= 128. Greedy on descending total
    degree. Returns (win, slot) per node."""
    tot = deg_pair.sum(1)
    order = np.argsort(-tot, kind="stable")
    loads = np.zeros((W, NP), np.int64)
    counts = np.zeros(W, np.int64)
    win = np.empty(PER, np.int64)
    slot = np.empty(PER, np.int64)
    for n in order:
        d = deg_pair[n]
        cand = np.max(loads + d[None, :], axis=1).astype(np.float64)
        cand[counts >= 128] = np.inf
        over = np.max(loads + d[None, :], axis=1) > 128
        feas = (~over) & (counts < 128)
        if feas.any():
            cand[over] = np.inf
        w = int(np.argmin(cand + counts * 1e-4))
        if counts[w] >= 128:
            raise RuntimeError("window overflow")
        win[n] = w
        slot[n] = counts[w]
        counts[w] += 1
        loads[w] += d
    if (loads > 128).any():
        raise RuntimeError(f"pair load overflow: {loads.max()}")
    return win, slot


def _prep(edge_index):
    src = edge_index[0].astype(np.int64)
    dst = edge_index[1].astype(np.int64)
    src_core = src // PER
    dst_core = dst // PER

    # per-core packing
    newpos = np.empty(N, np.int64)      # global padded row (core*NL + loc)
    for c in range(NC):
        mask_c = dst_core == c
        dloc = dst[mask_c] - c * PER
        sp = src_core[mask_c] // 2
        deg_pair = np.zeros((PER, NP), np.int64)
        np.add.at(deg_pair, (dloc, sp), 1)
        win, slot = _pack_windows(deg_pair)
        newpos[c * PER:(c + 1) * PER] = c * NL + win * 128 + slot

    s_row = newpos[src]                  # uag row of source
    d_row = newpos[dst]
    d_core = d_row // NL
    d_loc = d_row % NL

    gidx = np.zeros((NC, 128, 8 * T_TOTAL), np.int16)
    ohr = np.zeros((NC, 128, T_TOTAL * 128), np.float32)

    wk = np.searchsorted(np.cumsum([len(b) for b in BLOCKS]),
                         np.arange(W), side="right")  # window -> block
    wbase = np.array([BLOCKS[b][0] for b in wk[np.arange(W)]])

    for c in range(NC):
        m = d_core == c
        e_w = d_loc[m] // 128
        e_q = d_loc[m] % 128
        e_sr = s_row[m]
        e_p = (e_sr // NL) // 2
        e_rel = (e_sr - e_p * 2 * NL).astype(np.int64)  # idx within pair
        if e_rel.max() >= 2 * NL:
            raise RuntimeError("rel idx out of range")
        # rank within (w, p)
        key = e_w * NP + e_p
        order = np.argsort(key, kind="stable")
        ks = key[order]
        uniq, start = np.unique(ks, return_index=True)
        rank = np.arange(len(ks)) - start[np.searchsorted(uniq, ks)]
        if rank.max() >= 128:
            raise RuntimeError("tile overflow")
        ew, ep, eq = e_w[order], e_p[order], e_q[order]
        er = e_rel[order]
        eb = wk[ew]
        ewi = ew - wbase[ew]
        t_glob = np.asarray(_TB)[eb] + ep * np.array(
            [len(BLOCKS[b]) for b in eb]) + ewi
        # one-hot: [slot rank, t*128 + dst_q]
        ohr[c, rank, t_glob * 128 + eq] = 1.0
        # gather idx: group (b, p) list position = (wi*128 + rank)
        gpos = ewi * 128 + rank
        gbase = (np.asarray(_TB)[eb] + ep * np.array(
            [len(BLOCKS[b]) for b in eb])) * 128
        lin = gbase + gpos                 # linear idx slot over the layer
        col = lin // 16
        row = lin % 16
        for k in range(8):
            gidx[c, row + 16 * k, col] = er
    return {"newpos": newpos, "ohr": ohr.astype(BF16), "gidx": gidx}


def _build():
    if "nc" in _cached:
        return _cached["nc"]
    import concourse.bacc as bacc
    import concourse.mybir as mybir
    import concourse.tile as tile

    bf = mybir.dt.bfloat16
    f32 = mybir.dt.float32
    i16 = mybir.dt.int16

    nc = bacc.Bacc("TRN2", target_bir_lowering=False, debug=False,
                   num_devices=NC)
    x0T = nc.dram_tensor("x0T", [CH, NL], bf, kind="ExternalInput")
    x1T = nc.dram_tensor("x1T", [CH, NL], bf, kind="ExternalInput")
    lw = nc.dram_tensor("lw", [2, CH, DH], bf, kind="ExternalInput")
    lbT = nc.dram_tensor("lbT", [DH, 2], f32, kind="ExternalInput")
    wu = nc.dram_tensor("wu", [L, CH, CH], bf, kind="ExternalInput")
    wc = nc.dram_tensor("wc", [L, CH, CH], bf, kind="ExternalInput")
    ow = nc.dram_tensor("ow", [CH, C], bf, kind="ExternalInput")
    idn = nc.dram_tensor("idn", [128, 128], bf, kind="ExternalInput")
    ohr_d = nc.dram_tensor("ohr", [128, T_TOTAL * 128], bf,
                           kind="ExternalInput")
    gidx_d = nc.dram_tensor("gidx", [128, 8 * T_TOTAL], i16,
                            kind="ExternalInput")
    out_d = nc.dram_tensor("out", [NL, C], f32, kind="ExternalOutput")

    u_d = [nc.dram_tensor(f"u{i}", [NL, CH], bf, kind="Internal")
           for i in range(2)]
    uag = [nc.dram_tensor(f"uag{i}", [NC * NL, CH], bf, kind="Internal",
                          addr_space="Shared") for i in range(2)]

    relu = mybir.ActivationFunctionType.Relu

    with tile.TileContext(nc) as tc:
        with (
            tc.tile_pool(name="cst", bufs=1) as cst,
            tc.tile_pool(name="big", bufs=1) as big,
            tc.tile_pool(name="wp", bufs=2) as wp,
            tc.tile_pool(name="hp", bufs=3) as hp,
            tc.tile_pool(name="gp", bufs=2) as gp,
            tc.tile_pool(name="op", bufs=2) as op,
            tc.tile_pool(name="sp", bufs=4) as sp,
            tc.tile_pool(name="up", bufs=3) as up,
            tc.tile_pool(name="ps", bufs=3, space="PSUM") as ps,
            tc.tile_pool(name="psa", bufs=2, space="PSUM") as psa,
            tc.tile_pool(name="pst", bufs=2, space="PSUM") as pst,
        ):
            h0T_a = big.tile([128, NL], bf, tag="h0Ta")
            h0T_b = big.tile([128, NL], bf, tag="h0Tb")
            hT_a = big.tile([128, NL], bf, tag="hTa")
            hT_b = big.tile([128, NL], bf, tag="hTb")
            gidx_t = cst.tile([128, 8 * T_TOTAL], i16, tag="gidx")
            nc.sync.dma_start(out=gidx_t[:], in_=gidx_d[:])
            lbT_t = cst.tile([DH, 2], f32, tag="lbT")
            nc.sync.dma_start(out=lbT_t[:], in_=lbT[:])
            idn_t = cst.tile([128, 128], bf, tag="idn")
            nc.sync.dma_start(out=idn_t[:], in_=idn[:])

            # setup: h0T = relu(lw.T @ xT + b)  (feat-major halves)
            for view, (xT, h0T) in enumerate(((x0T, h0T_a), (x1T, h0T_b))):
                lwt = wp.tile([128, 2, DH], bf, tag="lwt")
                nc.sync.dma_start(
                    out=lwt[:],
                    in_=lw[view].rearrange("(k p) d -> p k d", p=128))
                for ch0 in range(0, NL, 512):
                    cw = min(512, NL - ch0)
                    xt = hp.tile([128, 2, 512], bf, tag="xt")
                    nc.sync.dma_start(
                        out=xt[:, :, :cw],
                        in_=xT[:, ch0:ch0 + cw].rearrange(
                            "(k p) n -> p k n", p=128))
                    p0 = ps.tile([DH, 512], f32, tag="mm")
                    for k in range(2):
                        nc.tensor.matmul(out=p0[:, :cw], lhsT=lwt[:, k, :],
                                         rhs=xt[:, k, :cw],
                                         start=(k == 0), stop=(k == 1))
                    nc.scalar.activation(
                        out=h0T[:, ch0:ch0 + cw], in_=p0[:, :cw], func=relu,
                        bias=lbT_t[:, view:view + 1])

            def emit_wload(l):
                wu_t = wp.tile([128, 2, CH], bf, tag="wu")
                wc_t = wp.tile([128, 2, CH], bf, tag="wc")
                nc.sync.dma_start(
                    out=wu_t[:], in_=wu[l].rearrange("(k p) d -> p k d", p=128))
                nc.sync.dma_start(
                    out=wc_t[:], in_=wc[l].rearrange("(k p) d -> p k d", p=128))
                return wu_t, wc_t

            def emit_dense(l, wu_t, ws):
                """u windows ws (a batch of <=4) for layer l; one DMA out."""
                ta = h0T_a if l == 0 else hT_a
                tb = h0T_b if l == 0 else hT_b
                nb = len(ws)
                us = up.tile([128, 4, CH], bf, tag="us")
                for j, w in enumerate(ws):
                    sl = slice(w * 128, (w + 1) * 128)
                    pu = ps.tile([128, CH], f32, tag="mm")
                    nc.tensor.matmul(out=pu[:], lhsT=ta[:, sl],
                                     rhs=wu_t[:, 0, :], start=True, stop=False)
                    nc.tensor.matmul(out=pu[:], lhsT=tb[:, sl],
                                     rhs=wu_t[:, 1, :], start=False, stop=True)
                    nc.vector.tensor_copy(out=us[:, j, :], in_=pu[:])
                w0 = ws[0]
                nc.sync.dma_start(
                    out=u_d[l % 2][w0 * 128:(w0 + nb) * 128, :].rearrange(
                        "(j p) c -> p j c", p=128),
                    in_=us[:, :nb, :])

            def emit_ag(l):
                nc.gpsimd.collective_compute(
                    "AllGather", mybir.AluOpType.bypass,
                    replica_groups=[list(range(NC))],
                    ins=[u_d[l % 2][:]], outs=[uag[l % 2][:]],
                )

            def emit_router(l, b, wc_t):
                """router for layer l, block b: gather + reduce + relu + hT."""
                wins = BLOCKS[b]
                nb = len(wins)
                tb = _TB[b]
                gat = gp.tile([128, NP * BW, CH], bf, tag="gat")
                ohr_t = op.tile([128, NP * BW * 128], bf, tag="ohr")
                nc.sync.dma_start(
                    out=ohr_t[:, :NP * nb * 128],
                    in_=ohr_d[:, tb * 128:(tb + NP * nb) * 128])
                for p in range(NP):
                    nidx = nb * 128
                    c0 = (tb + p * nb) * 8
                    nc.gpsimd.dma_gather(
                        out_ap=gat[:, p * nb:(p + 1) * nb, :],
                        in_ap=uag[l % 2][p * 2 * NL:(p + 1) * 2 * NL, :],
                        idxs_ap=gidx_t[:, c0:c0 + nidx // 16],
                        num_idxs=nidx, num_idxs_reg=nidx, elem_size=CH)
                for wi, w in enumerate(wins):
                    sl = slice(w * 128, (w + 1) * 128)
                    pa = psa.tile([128, CH], f32, tag="pa")
                    for p in range(NP):
                        tloc = p * nb + wi
                        nc.tensor.matmul(
                            out=pa[:], lhsT=ohr_t[:, tloc * 128:(tloc + 1) * 128],
                            rhs=gat[:, tloc, :], start=(p == 0), stop=False)
                    nc.tensor.matmul(out=pa[:], lhsT=h0T_a[:, sl],
                                     rhs=wc_t[:, 0, :], start=False, stop=False)
                    nc.tensor.matmul(out=pa[:], lhsT=h0T_b[:, sl],
                                     rhs=wc_t[:, 1, :], start=False, stop=True)
                    ho = sp.tile([128, CH], bf, tag="ho")
                    nc.scalar.activation(out=ho[:], in_=pa[:], func=relu)
                    for k, hT in enumerate((hT_a, hT_b)):
                        pt = pst.tile([128, 128], bf, tag="pt")
                        nc.tensor.transpose(
                            out=pt[:], in_=ho[:, k * 128:(k + 1) * 128],
                            identity=idn_t[:])
                        nc.vector.tensor_copy(out=hT[:, sl], in_=pt[:])

            WBATCH = [list(range(w, min(w + 4, W))) for w in range(0, W, 4)]

            wu_t, wc_t = emit_wload(0)
            for ws in WBATCH:
                emit_dense(0, wu_t, ws)
            emit_ag(0)
            for l in range(1, L):
                wu_n, wc_n = emit_wload(l)
                for b in range(NB):
                    emit_router(l - 1, b, wc_t)
                    wlo, whi = BLOCKS[b][0], BLOCKS[b][-1] + 1
                    for ws in [list(range(w, min(w + 4, whi)))
                               for w in range(wlo, whi, 4)]:
                        emit_dense(l, wu_n, ws)
                emit_ag(l)
                wu_t, wc_t = wu_n, wc_n
            for b in range(NB):
                emit_router(L - 1, b, wc_t)

            # final projection from resident hT
            owt = wp.tile([128, 2, C], bf, tag="ow")
            nc.sync.dma_start(
                out=owt[:], in_=ow.rearrange("(k p) d -> p k d", p=128))
            for ws in WBATCH:
                nb = len(ws)
                ot = up.tile([128, 4, C], f32, tag="ot")
                for j, w in enumerate(ws):
                    sl = slice(w * 128, (w + 1) * 128)
                    po = ps.tile([128, C], f32, tag="mm")
                    nc.tensor.matmul(out=po[:], lhsT=hT_a[:, sl],
                                     rhs=owt[:, 0, :], start=True, stop=False)
                    nc.tensor.matmul(out=po[:], lhsT=hT_b[:, sl],
                                     rhs=owt[:, 1, :], start=False, stop=True)
                    nc.vector.tensor_copy(out=ot[:, j, :], in_=po[:])
                w0 = ws[0]
                nc.sync.dma_start(
                    out=out_d[w0 * 128:(w0 + nb) * 128, :].rearrange(
                        "(j p) c -> p j c", p=128),
                    in_=ot[:, :nb, :])
    nc.compile()
    _cached["nc"] = nc
    return nc


def _in_maps(x0, x1, edge_index, lin_w, lin_b, gcn_w, out_w):
    ei = np.asarray(edge_index)
    key = ei.tobytes()[:256]
    if _cached.get("prep_key") != key:
        _cached["prep"] = _prep(ei)
        _cached["prep_key"] = key
    prep = _cached["prep"]
    newpos = prep["newpos"]

    betas = np.log(THETA / np.arange(1, L + 1, dtype=np.float32) + 1.0)
    eye = np.eye(CH, dtype=np.float32)
    wu = np.stack([0.9 * ((1 - b) * eye + b * np.asarray(gcn_w[i], np.float32))
                   for i, b in enumerate(betas)]).astype(BF16)
    wc = np.stack([0.1 * ((1 - b) * eye + b * np.asarray(gcn_w[i], np.float32))
                   for i, b in enumerate(betas)]).astype(BF16)

    x0p = np.zeros((NC * NL, CH), np.float32)
    x1p = np.zeros((NC * NL, CH), np.float32)
    x0p[newpos] = np.asarray(x0, np.float32)
    x1p[newpos] = np.asarray(x1, np.float32)
    x0p = x0p.astype(BF16)
    x1p = x1p.astype(BF16)

    idn = np.eye(128, dtype=np.float32).astype(BF16)

    maps = []
    for c in range(NC):
        maps.append({
            "x0T": np.ascontiguousarray(x0p[c * NL:(c + 1) * NL].T),
            "x1T": np.ascontiguousarray(x1p[c * NL:(c + 1) * NL].T),
            "lw": np.asarray(lin_w, np.float32).astype(BF16),
            "lbT": np.ascontiguousarray(np.asarray(lin_b, np.float32).T),
            "wu": wu,
            "wc": wc,
            "ow": np.asarray(out_w, np.float32).astype(BF16),
            "idn": idn,
            "ohr": prep["ohr"][c],
            "gidx": prep["gidx"][c],
        })
    return maps, newpos


def kernel(x0, x1, edge_index, lin_w, lin_b, gcn_w, out_w, out_b):
    from concourse import bass_utils

    maps, newpos = _in_maps(x0, x1, edge_index, lin_w, lin_b, gcn_w, out_w)
    nc = _build()
    res = bass_utils.run_bass_kernel_spmd(
        nc, maps, core_ids=list(range(NC)), trace=False)
    dev = np.concatenate([res.results[c]["out"] for c in range(NC)], axis=0)
    out = dev[newpos].astype(np.float32)
    out += np.asarray(out_b, np.float32)[None, :]
    return out


def _install_ntff_shim():
    """Register the axon NTFF profile hook if the image's antenv lacks it."""
    import contextlib
    import ctypes
    import sys
    import types
    try:
        import antenv.axon_hooks  # noqa: F401
        return
    except ImportError:
        pass
    try:
        lib = ctypes.CDLL("/opt/axon/libaxon_pjrt.so")
    except OSError:
        return
    if not hasattr(lib, "axon_start_nrt_profile"):
        return
    lib.axon_start_nrt_profile.argtypes = [ctypes.POINTER(ctypes.c_int64),
                                           ctypes.c_size_t]
    lib.axon_start_nrt_profile.restype = ctypes.c_int64
    lib.axon_stop_nrt_profile.argtypes = [ctypes.c_char_p]
    lib.axon_stop_nrt_profile.restype = ctypes.c_int64

    @contextlib.contextmanager
    def _hook(output_dir, device_ids):
        import jax
        jax.devices()
        if device_ids:
            ids = (ctypes.c_int64 * len(device_ids))(*device_ids)
            rc = lib.axon_start_nrt_profile(ids, len(device_ids))
        else:
            rc = lib.axon_start_nrt_profile(None, 0)
        if rc != 0:
            raise RuntimeError(f"axon_start_nrt_profile rc={rc}")
        try:
            yield
        finally:
            lib.axon_stop_nrt_profile(str(output_dir).encode())

    mod = types.ModuleType("antenv.axon_hooks")
    state = {"hook": _hook}
    mod.get_axon_ntff_profile_hook = lambda: state["hook"]
    mod.set_axon_ntff_profile_hook = lambda h: state.update(hook=h)
    sys.modules["antenv.axon_hooks"] = mod
    try:
        import antenv
        antenv.axon_hooks = mod
    except ImportError:
        pass


def profile(inputs):
    """Run once with NTFF tracing, return exec_time_ns (for test.py)."""
    from concourse import bass_utils

    _install_ntff_shim()

    maps, _ = _in_maps(inputs["x0"], inputs["x1"], inputs["edge_index"],
                       inputs["lin_w"], inputs["lin_b"], inputs["gcn_w"],
                       inputs["out_w"])
    nc = _build()
    res = bass_utils.run_bass_kernel_spmd(
        nc, maps, core_ids=list(range(NC)), trace=True)
    return res.exec_time_ns


# revision 4
# speedup vs baseline: 1.4469x; 1.4469x over previous
"""LAGCNII full-device kernel for 8 TRN2 NeuronCores (v3).

Node-sharded (12544 padded rows/core, 98 windows of 128). Per layer l
(W'_l = (1-b)I + b*W_l):
  u = h @ (0.9 W'_l)        dense pass, hT kept resident in SBUF
  AllGather(u) -> uag       one collective per layer, rank-major layout
  router: per dst window w the incoming source rows are pre-grouped by
    source HBM-pair (uag rows are pair-contiguous, so int16 dma_gather
    indices address one pair region); gathered tiles reduce into PSUM
    via one-hot matmuls; the c = h0 @ (0.1 W'_l) term accumulates into
    the same PSUM; relu -> h; PE-transpose -> hT (SBUF, single buffer)
Tiles are a uniform (window x pair) grid: exactly 1 gather tile per
(w, p), 4 per window, 392 per layer, so the program is identical across
cores (SPMD) and only the gather indices / one-hot payloads differ.
Gathers are batched with dma_gather per (8-window block, pair):
52 calls/layer instead of 302 indirect DMAs. One-hots stream from DRAM
pre-transposed ([128 slot, tile*128+dst] layout) in 13 big DMAs/layer.
h never round-trips DRAM. Output fp32.
"""
import numpy as np
import ml_dtypes

BF16 = ml_dtypes.bfloat16

N = 100000
E = 300000
PER = 12500
NL = 12544           # 98*128
W = 98
CH = 256
DH = 128
C = 40
L = 8
NC = 8
NP = 4               # HBM pairs
THETA = 0.5

BW = 8                                    # windows per block
BLOCKS = [list(range(b, min(b + BW, W))) for b in range(0, W, BW)]
NB = len(BLOCKS)                          # 13
T_TOTAL = NP * W                          # 392 tiles per layer
# global tile index: blocks outer, then pair, then window-within-block
_TB = []                                  # per block: base tile index
_t = 0
for _b in BLOCKS:
    _TB.append(_t)
    _t += NP * len(_b)


def tile_index(b, p, wi):
    return _TB[b] + p * len(BLOCKS[b]) + wi


_cached = {}


def _pack_windows(deg_pair):
    """Assign PER nodes to W windows of exactly 128 slots, keeping the
    per-(window, pair) in-degree load <= 128. Greedy on descending total
    degree. Returns (win, slot) per node."""
    tot = deg_pair.sum(1)
    order = np.argsort(-tot, kind="stable")
    loads = np.zeros((W, NP), np.int64)
    counts = np.zeros(W, np.int64)
    win = np.empty(PER, np.int64)
    slot = np.empty(PER, np.int64)
    for n in order:
        d = deg_pair[n]
        cand = np.max(loads + d[None, :], axis=1).astype(np.float64)
        cand[counts >= 128] = np.inf
        over = np.max(loads + d[None, :], axis=1) > 128
        feas = (~over) & (counts < 128)
        if feas.any():
            cand[over] = np.inf
        w = int(np.argmin(cand + counts * 1e-4))
        if counts[w] >= 128:
            raise RuntimeError("window overflow")
        win[n] = w
        slot[n] = counts[w]
        counts[w] += 1
        loads[w] += d
    if (loads > 128).any():
        raise RuntimeError(f"pair load overflow: {loads.max()}")
    return win, slot


def _prep(edge_index):
    src = edge_index[0].astype(np.int64)
    dst = edge_index[1].astype(np.int64)
    src_core = src // PER
    dst_core = dst // PER

    # per-core packing
    newpos = np.empty(N, np.int64)      # global padded row (core*NL + loc)
    for c in range(NC):
        mask_c = dst_core == c
        dloc = dst[mask_c] - c * PER
        sp = src_core[mask_c] // 2
        deg_pair = np.zeros((PER, NP), np.int64)
        np.add.at(deg_pair, (dloc, sp), 1)
        win, slot = _pack_windows(deg_pair)
        newpos[c * PER:(c + 1) * PER] = c * NL + win * 128 + slot

    s_row = newpos[src]                  # uag row of source
    d_row = newpos[dst]
    d_core = d_row // NL
    d_loc = d_row % NL

    gidx = np.zeros((NC, 128, 8 * T_TOTAL), np.int16)
    ohr = np.zeros((NC, 128, T_TOTAL * 128), np.float32)

    wk = np.searchsorted(np.cumsum([len(b) for b in BLOCKS]),
                         np.arange(W), side="right")  # window -> block
    wbase = np.array([BLOCKS[b][0] for b in wk[np.arange(W)]])

    for c in range(NC):
        m = d_core == c
        e_w = d_loc[m] // 128
        e_q = d_loc[m] % 128
        e_sr = s_row[m]
        e_p = (e_sr // NL) // 2
        e_rel = (e_sr - e_p * 2 * NL).astype(np.int64)  # idx within pair
        if e_rel.max() >= 2 * NL:
            raise RuntimeError("rel idx out of range")
        # rank within (w, p)
        key = e_w * NP + e_p
        order = np.argsort(key, kind="stable")
        ks = key[order]
        uniq, start = np.unique(ks, return_index=True)
        rank = np.arange(len(ks)) - start[np.searchsorted(uniq, ks)]
        if rank.max() >= 128:
            raise RuntimeError("tile overflow")
        ew, ep, eq = e_w[order], e_p[order], e_q[order]
        er = e_rel[order]
        eb = wk[ew]
        ewi = ew - wbase[ew]
        t_glob = np.asarray(_TB)[eb] + ep * np.array(
            [len(BLOCKS[b]) for b in eb]) + ewi
        # one-hot: [slot rank, t*128 + dst_q]
        ohr[c, rank, t_glob * 128 + eq] = 1.0
        # gather idx: group (b, p) list position = (wi*128 + rank)
        gpos = ewi * 128 + rank
        gbase = (np.asarray(_TB)[eb] + ep * np.array(
            [len(BLOCKS[b]) for b in eb])) * 128
        lin = gbase + gpos                 # linear idx slot over the layer
        col = lin // 16
        row = lin % 16
        for k in range(8):
            gidx[c, row + 16 * k, col] = er
    return {"newpos": newpos, "ohr": ohr.astype(BF16), "gidx": gidx}


def _build():
    if "nc" in _cached:
        return _cached["nc"]
    import concourse.bacc as bacc
    import concourse.mybir as mybir
    import concourse.tile as tile

    bf = mybir.dt.bfloat16
    f32 = mybir.dt.float32
    i16 = mybir.dt.int16

    nc = bacc.Bacc("TRN2", target_bir_lowering=False, debug=False,
                   num_devices=NC, num_swdge_queues=4)
    x0T = nc.dram_tensor("x0T", [CH, NL], bf, kind="ExternalInput")
    x1T = nc.dram_tensor("x1T", [CH, NL], bf, kind="ExternalInput")
    lw = nc.dram_tensor("lw", [2, CH, DH], bf, kind="ExternalInput")
    lbT = nc.dram_tensor("lbT", [DH, 2], f32, kind="ExternalInput")
    wu = nc.dram_tensor("wu", [L, CH, CH], bf, kind="ExternalInput")
    wc = nc.dram_tensor("wc", [L, CH, CH], bf, kind="ExternalInput")
    ow = nc.dram_tensor("ow", [CH, C], bf, kind="ExternalInput")
    idn = nc.dram_tensor("idn", [128, 128], bf, kind="ExternalInput")
    ohr_d = nc.dram_tensor("ohr", [128, T_TOTAL * 128], bf,
                           kind="ExternalInput")
    gidx_d = nc.dram_tensor("gidx", [128, 8 * T_TOTAL], i16,
                            kind="ExternalInput")
    out_d = nc.dram_tensor("out", [NL, C], f32, kind="ExternalOutput")

    u_d = [nc.dram_tensor(f"u{i}", [NL, CH], bf, kind="Internal")
           for i in range(2)]
    uag = [nc.dram_tensor(f"uag{i}", [NC * NL, CH], bf, kind="Internal",
                          addr_space="Shared") for i in range(2)]

    relu = mybir.ActivationFunctionType.Relu

    with tile.TileContext(nc) as tc:
        with (
            tc.tile_pool(name="cst", bufs=1) as cst,
            tc.tile_pool(name="big", bufs=1) as big,
            tc.tile_pool(name="wp", bufs=2) as wp,
            tc.tile_pool(name="hp", bufs=3) as hp,
            tc.tile_pool(name="gp", bufs=2) as gp,
            tc.tile_pool(name="op", bufs=2) as op,
            tc.tile_pool(name="sp", bufs=4) as sp,
            tc.tile_pool(name="up", bufs=3) as up,
            tc.tile_pool(name="ps", bufs=3, space="PSUM") as ps,
            tc.tile_pool(name="psa", bufs=2, space="PSUM") as psa,
            tc.tile_pool(name="pst", bufs=2, space="PSUM") as pst,
        ):
            h0T_a = big.tile([128, NL], bf, tag="h0Ta")
            h0T_b = big.tile([128, NL], bf, tag="h0Tb")
            hT_a = big.tile([128, NL], bf, tag="hTa")
            hT_b = big.tile([128, NL], bf, tag="hTb")
            gidx_t = cst.tile([128, 8 * T_TOTAL], i16, tag="gidx")
            nc.sync.dma_start(out=gidx_t[:], in_=gidx_d[:])
            lbT_t = cst.tile([DH, 2], f32, tag="lbT")
            nc.sync.dma_start(out=lbT_t[:], in_=lbT[:])
            idn_t = cst.tile([128, 128], bf, tag="idn")
            nc.sync.dma_start(out=idn_t[:], in_=idn[:])

            # setup: h0T = relu(lw.T @ xT + b)  (feat-major halves)
            for view, (xT, h0T) in enumerate(((x0T, h0T_a), (x1T, h0T_b))):
                lwt = wp.tile([128, 2, DH], bf, tag="lwt")
                nc.sync.dma_start(
                    out=lwt[:],
                    in_=lw[view].rearrange("(k p) d -> p k d", p=128))
                for ch0 in range(0, NL, 512):
                    cw = min(512, NL - ch0)
                    xt = hp.tile([128, 2, 512], bf, tag="xt")
                    nc.sync.dma_start(
                        out=xt[:, :, :cw],
                        in_=xT[:, ch0:ch0 + cw].rearrange(
                            "(k p) n -> p k n", p=128))
                    p0 = ps.tile([DH, 512], f32, tag="mm")
                    for k in range(2):
                        nc.tensor.matmul(out=p0[:, :cw], lhsT=lwt[:, k, :],
                                         rhs=xt[:, k, :cw],
                                         start=(k == 0), stop=(k == 1))
                    nc.scalar.activation(
                        out=h0T[:, ch0:ch0 + cw], in_=p0[:, :cw], func=relu,
                        bias=lbT_t[:, view:view + 1])

            def emit_wload(l):
                wu_t = wp.tile([128, 2, CH], bf, tag="wu")
                wc_t = wp.tile([128, 2, CH], bf, tag="wc")
                nc.sync.dma_start(
                    out=wu_t[:], in_=wu[l].rearrange("(k p) d -> p k d", p=128))
                nc.sync.dma_start(
                    out=wc_t[:], in_=wc[l].rearrange("(k p) d -> p k d", p=128))
                return wu_t, wc_t

            def emit_dense(l, wu_t, ws):
                """u windows ws (a batch of <=4) for layer l; one DMA out."""
                ta = h0T_a if l == 0 else hT_a
                tb = h0T_b if l == 0 else hT_b
                nb = len(ws)
                us = up.tile([128, 4, CH], bf, tag="us")
                for j, w in enumerate(ws):
                    sl = slice(w * 128, (w + 1) * 128)
                    pu = ps.tile([128, CH], f32, tag="mm")
                    nc.tensor.matmul(out=pu[:], lhsT=ta[:, sl],
                                     rhs=wu_t[:, 0, :], start=True, stop=False)
                    nc.tensor.matmul(out=pu[:], lhsT=tb[:, sl],
                                     rhs=wu_t[:, 1, :], start=False, stop=True)
                    nc.vector.tensor_copy(out=us[:, j, :], in_=pu[:])
                w0 = ws[0]
                nc.sync.dma_start(
                    out=u_d[l % 2][w0 * 128:(w0 + nb) * 128, :].rearrange(
                        "(j p) c -> p j c", p=128),
                    in_=us[:, :nb, :])

            def emit_ag(l):
                nc.gpsimd.collective_compute(
                    "AllGather", mybir.AluOpType.bypass,
                    replica_groups=[list(range(NC))],
                    ins=[u_d[l % 2][:]], outs=[uag[l % 2][:]],
                )

            def emit_router(l, b, wc_t):
                """router for layer l, block b: gather + reduce + relu + hT."""
                wins = BLOCKS[b]
                nb = len(wins)
                tb = _TB[b]
                gat = gp.tile([128, NP * BW, CH], bf, tag="gat")
                ohr_t = op.tile([128, NP * BW * 128], bf, tag="ohr")
                nc.sync.dma_start(
                    out=ohr_t[:, :NP * nb * 128],
                    in_=ohr_d[:, tb * 128:(tb + NP * nb) * 128])
                for p in range(NP):
                    nidx = nb * 128
                    c0 = (tb + p * nb) * 8
                    nc.gpsimd.dma_gather(
                        out_ap=gat[:, p * nb:(p + 1) * nb, :],
                        in_ap=uag[l % 2][p * 2 * NL:(p + 1) * 2 * NL, :],
                        idxs_ap=gidx_t[:, c0:c0 + nidx // 16],
                        num_idxs=nidx, num_idxs_reg=nidx, elem_size=CH,
                        queue_num=p)
                for wi, w in enumerate(wins):
                    sl = slice(w * 128, (w + 1) * 128)
                    pa = psa.tile([128, CH], f32, tag="pa")
                    for p in range(NP):
                        tloc = p * nb + wi
                        nc.tensor.matmul(
                            out=pa[:], lhsT=ohr_t[:, tloc * 128:(tloc + 1) * 128],
                            rhs=gat[:, tloc, :], start=(p == 0), stop=False)
                    nc.tensor.matmul(out=pa[:], lhsT=h0T_a[:, sl],
                                     rhs=wc_t[:, 0, :], start=False, stop=False)
                    nc.tensor.matmul(out=pa[:], lhsT=h0T_b[:, sl],
                                     rhs=wc_t[:, 1, :], start=False, stop=True)
                    ho = sp.tile([128, CH], bf, tag="ho")
                    nc.scalar.activation(out=ho[:], in_=pa[:], func=relu)
                    for k, hT in enumerate((hT_a, hT_b)):
                        pt = pst.tile([128, 128], bf, tag="pt")
                        nc.tensor.transpose(
                            out=pt[:], in_=ho[:, k * 128:(k + 1) * 128],
                            identity=idn_t[:])
                        nc.vector.tensor_copy(out=hT[:, sl], in_=pt[:])

            WBATCH = [list(range(w, min(w + 4, W))) for w in range(0, W, 4)]

            wu_t, wc_t = emit_wload(0)
            for ws in WBATCH:
                emit_dense(0, wu_t, ws)
            emit_ag(0)
            for l in range(1, L):
                wu_n, wc_n = emit_wload(l)
                for b in range(NB):
                    emit_router(l - 1, b, wc_t)
                    wlo, whi = BLOCKS[b][0], BLOCKS[b][-1] + 1
                    for ws in [list(range(w, min(w + 4, whi)))
                               for w in range(wlo, whi, 4)]:
                        emit_dense(l, wu_n, ws)
                emit_ag(l)
                wu_t, wc_t = wu_n, wc_n
            for b in range(NB):
                emit_router(L - 1, b, wc_t)

            # final projection from resident hT
            owt = wp.tile([128, 2, C], bf, tag="ow")
            nc.sync.dma_start(
                out=owt[:], in_=ow.rearrange("(k p) d -> p k d", p=128))
            for ws in WBATCH:
                nb = len(ws)
                ot = up.tile([128, 4, C], f32, tag="ot")
                for j, w in enumerate(ws):
                    sl = slice(w * 128, (w + 1) * 128)
                    po = ps.tile([128, C], f32, tag="mm")
                    nc.tensor.matmul(out=po[:], lhsT=hT_a[:, sl],
                                     rhs=owt[:, 0, :], start=True, stop=False)
                    nc.tensor.matmul(out=po[:], lhsT=hT_b[:, sl],
                                     rhs=owt[:, 1, :], start=False, stop=True)
                    nc.vector.tensor_copy(out=ot[:, j, :], in_=po[:])
                w0 = ws[0]
                nc.sync.dma_start(
                    out=out_d[w0 * 128:(w0 + nb) * 128, :].rearrange(
                        "(j p) c -> p j c", p=128),
                    in_=ot[:, :nb, :])
    nc.compile()
    _cached["nc"] = nc
    return nc


def _in_maps(x0, x1, edge_index, lin_w, lin_b, gcn_w, out_w):
    ei = np.asarray(edge_index)
    key = ei.tobytes()[:256]
    if _cached.get("prep_key") != key:
        _cached["prep"] = _prep(ei)
        _cached["prep_key"] = key
    prep = _cached["prep"]
    newpos = prep["newpos"]

    betas = np.log(THETA / np.arange(1, L + 1, dtype=np.float32) + 1.0)
    eye = np.eye(CH, dtype=np.float32)
    wu = np.stack([0.9 * ((1 - b) * eye + b * np.asarray(gcn_w[i], np.float32))
                   for i, b in enumerate(betas)]).astype(BF16)
    wc = np.stack([0.1 * ((1 - b) * eye + b * np.asarray(gcn_w[i], np.float32))
                   for i, b in enumerate(betas)]).astype(BF16)

    x0p = np.zeros((NC * NL, CH), np.float32)
    x1p = np.zeros((NC * NL, CH), np.float32)
    x0p[newpos] = np.asarray(x0, np.float32)
    x1p[newpos] = np.asarray(x1, np.float32)
    x0p = x0p.astype(BF16)
    x1p = x1p.astype(BF16)

    idn = np.eye(128, dtype=np.float32).astype(BF16)

    maps = []
    for c in range(NC):
        maps.append({
            "x0T": np.ascontiguousarray(x0p[c * NL:(c + 1) * NL].T),
            "x1T": np.ascontiguousarray(x1p[c * NL:(c + 1) * NL].T),
            "lw": np.asarray(lin_w, np.float32).astype(BF16),
            "lbT": np.ascontiguousarray(np.asarray(lin_b, np.float32).T),
            "wu": wu,
            "wc": wc,
            "ow": np.asarray(out_w, np.float32).astype(BF16),
            "idn": idn,
            "ohr": prep["ohr"][c],
            "gidx": prep["gidx"][c],
        })
    return maps, newpos


def kernel(x0, x1, edge_index, lin_w, lin_b, gcn_w, out_w, out_b):
    from concourse import bass_utils

    maps, newpos = _in_maps(x0, x1, edge_index, lin_w, lin_b, gcn_w, out_w)
    nc = _build()
    res = bass_utils.run_bass_kernel_spmd(
        nc, maps, core_ids=list(range(NC)), trace=False)
    dev = np.concatenate([res.results[c]["out"] for c in range(NC)], axis=0)
    out = dev[newpos].astype(np.float32)
    out += np.asarray(out_b, np.float32)[None, :]
    return out


def _install_ntff_shim():
    """Register the axon NTFF profile hook if the image's antenv lacks it."""
    import contextlib
    import ctypes
    import sys
    import types
    try:
        import antenv.axon_hooks  # noqa: F401
        return
    except ImportError:
        pass
    try:
        lib = ctypes.CDLL("/opt/axon/libaxon_pjrt.so")
    except OSError:
        return
    if not hasattr(lib, "axon_start_nrt_profile"):
        return
    lib.axon_start_nrt_profile.argtypes = [ctypes.POINTER(ctypes.c_int64),
                                           ctypes.c_size_t]
    lib.axon_start_nrt_profile.restype = ctypes.c_int64
    lib.axon_stop_nrt_profile.argtypes = [ctypes.c_char_p]
    lib.axon_stop_nrt_profile.restype = ctypes.c_int64

    @contextlib.contextmanager
    def _hook(output_dir, device_ids):
        import jax
        jax.devices()
        if device_ids:
            ids = (ctypes.c_int64 * len(device_ids))(*device_ids)
            rc = lib.axon_start_nrt_profile(ids, len(device_ids))
        else:
            rc = lib.axon_start_nrt_profile(None, 0)
        if rc != 0:
            raise RuntimeError(f"axon_start_nrt_profile rc={rc}")
        try:
            yield
        finally:
            lib.axon_stop_nrt_profile(str(output_dir).encode())

    mod = types.ModuleType("antenv.axon_hooks")
    state = {"hook": _hook}
    mod.get_axon_ntff_profile_hook = lambda: state["hook"]
    mod.set_axon_ntff_profile_hook = lambda h: state.update(hook=h)
    sys.modules["antenv.axon_hooks"] = mod
    try:
        import antenv
        antenv.axon_hooks = mod
    except ImportError:
        pass


def profile(inputs):
    """Run once with NTFF tracing, return exec_time_ns (for test.py)."""
    from concourse import bass_utils

    _install_ntff_shim()

    maps, _ = _in_maps(inputs["x0"], inputs["x1"], inputs["edge_index"],
                       inputs["lin_w"], inputs["lin_b"], inputs["gcn_w"],
                       inputs["out_w"])
    nc = _build()
    res = bass_utils.run_bass_kernel_spmd(
        nc, maps, core_ids=list(range(NC)), trace=True)
    return res.exec_time_ns
